# revision 3
# baseline (speedup 1.0000x reference)
"""Trainium2 Bass kernel for nn_CSG2A_net (gnn_message_passing).

Math (algebraically identical to the reference, never materializes the
[B,G,G] score tensor):
  CCE:  h = relu(node_feat @ W1); w = adj*exp(-dist)
        g[b,m] = sum_n mask[b,n] * w[b,n,m]
        pooled[b,d] = (sum_m g[b,m] h[b,m,d]) / clip(sum_n mask[b,n], 1)
        comp = pooled @ W2 + dose @ w_dose + time @ w_time
  score.sum(-1)[b,g] = q[b,g,:] . (sum_k q[b,k,:]) / sqrt(H)
    with q[b,g,:] = b_gex[b,g] w_gex[g,:] + comp[b,g] w_comp[g,:]
    so  u = b_gex @ w_gex + comp @ w_comp          [B,H]
        A = u @ w_gex.T ; C = u @ w_comp.T         [B,G]
        ssum = (b_gex*A + comp*C) / sqrt(H)
  pred = b_gex * (ssum + ppi_adj.sum(-1))
  out  = relu(LN(pred)) @ W_ff

Sharding: data-parallel over batch across 8 cores (8 samples each);
all weights replicated.  On-chip layout is gene-major ("transposed",
[G-tile partitions x batch free]) so every matmul contracts on the
partition dim without reformatting.
"""

import numpy as np

import concourse.bass as bass
import concourse.mybir as mybir
import concourse.tile as tile
from concourse.bass_utils import run_bass_kernel_spmd
from concourse.masks import make_identity

F32 = mybir.dt.float32
AF = mybir.ActivationFunctionType

G, H, NA, FEAT, CH = 978, 128, 50, 34, 64
B, NCORES = 64, 8
BL = B // NCORES  # per-core batch
LN_EPS = 1e-5
# gene-dim tiles: 7 x 128 + 82
GTS = [(i * 128, 128) for i in range(7)] + [(896, 82)]
NGT = len(GTS)

_DMA_ZERO_WAIT = ("InstDMACopy", "InstDMATransposeAnt", "InstTriggeredCopy")


def _split_excess_waits(nc):
    """walrus in this container accepts at most 1 inline sync-wait per
    instruction (0 for DMA).  Move excess waits onto same-engine nops
    inserted immediately before the overloaded instruction."""

    def make_nop(engine):
        bi = nc.engines[engine].nop(nofuse=True)
        ins = bi.ins
        lst = nc.cur_bb.bb.instructions
        assert lst[-1] is ins
        lst.pop()
        return ins

    for bb in nc.main_func.blocks:
        lst = bb.instructions
        i = 0
        while i < len(lst):
            ins = lst[i]
            si = getattr(ins, "sync_info", None)
            waits = list(si.on_wait) if (si and si.on_wait) else []
            limit = 0 if type(ins).__name__ in _DMA_ZERO_WAIT else 1
            if len(waits) > limit:
                keep = waits[len(waits) - limit:] if limit else []
                excess = waits[: len(waits) - limit]
                si.on_wait = keep
                pos = i
                for w in excess:
                    nop = make_nop(ins.engine)
                    nop.sync_info = mybir.SyncInfo(on_wait=[w], on_update=[])
                    lst.insert(pos, nop)
                    pos += 1
                    i += 1
            i += 1


def build_nc():
    nc = bass.Bass()

    # ---- kernel I/O (per-core shapes) ----
    b_gex = nc.dram_tensor("b_gex", [BL, G], F32, kind="ExternalInput")
    node_feat = nc.dram_tensor("node_feat", [BL, NA, FEAT], F32, kind="ExternalInput")
    mask = nc.dram_tensor("mask", [BL, NA], F32, kind="ExternalInput")
    adj = nc.dram_tensor("adj_matrix", [BL, NA, NA], F32, kind="ExternalInput")
    dist = nc.dram_tensor("dist_matrix", [BL, NA, NA], F32, kind="ExternalInput")
    dose = nc.dram_tensor("dose", [BL, 1], F32, kind="ExternalInput")
    time_in = nc.dram_tensor("time", [BL, 1], F32, kind="ExternalInput")
    ppi = nc.dram_tensor("ppi_adj", [G, G], F32, kind="ExternalInput")
    w_gex = nc.dram_tensor("w_gex", [G, H], F32, kind="ExternalInput")
    w_comp = nc.dram_tensor("w_comp", [G, H], F32, kind="ExternalInput")
    W1 = nc.dram_tensor("W1", [FEAT, CH], F32, kind="ExternalInput")
    W2 = nc.dram_tensor("W2", [CH, G], F32, kind="ExternalInput")
    w_dose = nc.dram_tensor("w_dose", [1, G], F32, kind="ExternalInput")
    w_time = nc.dram_tensor("w_time", [1, G], F32, kind="ExternalInput")
    ln_gamma = nc.dram_tensor("ln_gamma", [G], F32, kind="ExternalInput")
    ln_beta = nc.dram_tensor("ln_beta", [G], F32, kind="ExternalInput")
    W_ff = nc.dram_tensor("W_ff", [G, G], F32, kind="ExternalInput")

    out_pred = nc.dram_tensor("out_pred", [BL, G], F32, kind="ExternalOutput")
    out_comp = nc.dram_tensor("out_comp", [BL, G], F32, kind="ExternalOutput")

    inv_sqrt_h = 1.0 / float(np.sqrt(H))

    with tile.TileContext(nc) as tc:
        with (
            tc.tile_pool(name="const", bufs=1) as const,
            tc.tile_pool(name="sb", bufs=1) as sb,
            tc.tile_pool(name="big", bufs=2) as big,
            tc.tile_pool(name="pacc", bufs=1, space="PSUM") as pacc,
            tc.tile_pool(name="pcyc", bufs=4, space="PSUM") as pcyc,
        ):
            ident = const.tile([128, 128], F32)
            make_identity(nc, ident[:])
            ones_col = const.tile([128, 1], F32)   # lhsT for col-sums
            nc.vector.memset(ones_col[:], 1.0)
            ones_row = const.tile([1, 128], F32)   # lhsT for partition-bcast
            nc.vector.memset(ones_row[:], 1.0)
            eps_t = const.tile([1, 1], F32)
            nc.vector.memset(eps_t[:], LN_EPS)

            _cyc_n = [0]

            def cyc(shape):
                _cyc_n[0] += 1
                return pcyc.tile(shape, F32, tag="cyc", name=f"cyc{_cyc_n[0]}")

            # ================= CCE =================
            nfT = sb.tile([FEAT, BL, NA], F32)
            nc.sync.dma_start(out=nfT[:], in_=node_feat[:, :, :].rearrange("b n f -> f b n"))
            W1_sb = sb.tile([FEAT, CH], F32)
            nc.sync.dma_start(out=W1_sb[:], in_=W1[:, :])

            hT_ps = cyc([CH, BL * NA])
            nc.tensor.matmul(hT_ps[:], W1_sb[:], nfT[:].rearrange("f b n -> f (b n)"),
                             start=True, stop=True)
            hT = sb.tile([CH, BL, NA], F32)
            nc.scalar.activation(hT[:].rearrange("d b n -> d (b n)"), hT_ps[:], AF.Relu)

            adjT = sb.tile([NA, BL, NA], F32)
            nc.sync.dma_start(out=adjT[:], in_=adj[:, :, :].rearrange("b n m -> n b m"))
            distT = sb.tile([NA, BL, NA], F32)
            nc.sync.dma_start(out=distT[:], in_=dist[:, :, :].rearrange("b n m -> n b m"))
            wmsg = sb.tile([NA, BL, NA], F32)
            nc.scalar.activation(wmsg[:], distT[:], AF.Exp, scale=-1.0)
            nc.vector.tensor_mul(wmsg[:], wmsg[:], adjT[:])

            maskT = sb.tile([NA, BL], F32)
            nc.sync.dma_start(out=maskT[:], in_=mask[:, :].rearrange("b n -> n b"))

            g_ps = cyc([1, BL * NA])
            for b in range(BL):
                nc.tensor.matmul(g_ps[:, b * NA:(b + 1) * NA],
                                 maskT[:, b:b + 1], wmsg[:, b, :],
                                 start=True, stop=True)
            gb_ps = cyc([CH, BL * NA])
            g_sb = sb.tile([1, BL * NA], F32)
            nc.vector.tensor_copy(g_sb[:], g_ps[:])
            nc.tensor.matmul(gb_ps[:], ones_row[:1, :CH], g_sb[:], start=True, stop=True)

            prod = sb.tile([CH, BL, NA], F32)
            nc.vector.tensor_mul(prod[:].rearrange("d b n -> d (b n)"),
                                 hT[:].rearrange("d b n -> d (b n)"), gb_ps[:])
            pooled_raw = sb.tile([CH, BL], F32)
            nc.vector.tensor_reduce(pooled_raw[:], prod[:], mybir.AxisListType.X,
                                    mybir.AluOpType.add)

            ms_ps = cyc([1, BL])
            nc.tensor.matmul(ms_ps[:], ones_col[:NA, :], maskT[:], start=True, stop=True)
            ms_sb = sb.tile([1, BL], F32)
            nc.vector.tensor_scalar_max(ms_sb[:], ms_ps[:], 1.0)
            rms = sb.tile([1, BL], F32)
            nc.vector.reciprocal(rms[:], ms_sb[:])
            rb_ps = cyc([CH, BL])
            nc.tensor.matmul(rb_ps[:], ones_row[:1, :CH], rms[:], start=True, stop=True)
            pooledT = sb.tile([CH, BL], F32)
            nc.vector.tensor_mul(pooledT[:], pooled_raw[:], rb_ps[:])

            # comp.T per gene tile
            W2_sb = sb.tile([CH, G], F32)
            nc.sync.dma_start(out=W2_sb[:], in_=W2[:, :])
            wdose_sb = sb.tile([1, G], F32)
            nc.sync.dma_start(out=wdose_sb[:], in_=w_dose[:, :])
            wtime_sb = sb.tile([1, G], F32)
            nc.sync.dma_start(out=wtime_sb[:], in_=w_time[:, :])
            doseT = sb.tile([1, BL], F32)
            nc.sync.dma_start(out=doseT[:], in_=dose[:, :].rearrange("b o -> o b"))
            timeT = sb.tile([1, BL], F32)
            nc.sync.dma_start(out=timeT[:], in_=time_in[:, :].rearrange("b o -> o b"))

            compT = sb.tile([128, NGT, BL], F32)  # [p, gt, b]
            comp_out = sb.tile([BL, G], F32)
            for gt, (gs, gn) in enumerate(GTS):
                cT_ps = cyc([128, BL])
                nc.tensor.matmul(cT_ps[:gn, :], W2_sb[:, gs:gs + gn], pooledT[:],
                                 start=True, stop=False)
                nc.tensor.matmul(cT_ps[:gn, :], wdose_sb[:1, gs:gs + gn], doseT[:],
                                 start=False, stop=False)
                nc.tensor.matmul(cT_ps[:gn, :], wtime_sb[:1, gs:gs + gn], timeT[:],
                                 start=False, stop=True)
                nc.scalar.copy(compT[:gn, gt, :], cT_ps[:gn, :])
                c8_ps = cyc([BL, 128])
                nc.tensor.transpose(c8_ps[:, :gn], compT[:gn, gt, :], ident[:gn, :gn])
                nc.scalar.copy(comp_out[:, gs:gs + gn], c8_ps[:, :gn])
            nc.sync.dma_start(out=out_comp[:, :], in_=comp_out[:])

            # ================= attention-sum =================
            bgT = sb.tile([128, NGT, BL], F32)
            for gt, (gs, gn) in enumerate(GTS):
                nc.sync.dma_start(out=bgT[:gn, gt, :],
                                  in_=b_gex[:, gs:gs + gn].rearrange("b g -> g b"))
            wg_sb = sb.tile([128, NGT, H], F32)
            wc_sb = sb.tile([128, NGT, H], F32)
            for gt, (gs, gn) in enumerate(GTS):
                nc.sync.dma_start(out=wg_sb[:gn, gt, :], in_=w_gex[gs:gs + gn, :])
                nc.sync.dma_start(out=wc_sb[:gn, gt, :], in_=w_comp[gs:gs + gn, :])

            u_ps = pacc.tile([H, BL], F32, tag="u")
            for gt, (gs, gn) in enumerate(GTS):
                nc.tensor.matmul(u_ps[:], wg_sb[:gn, gt, :], bgT[:gn, gt, :],
                                 start=(gt == 0), stop=False)
            for gt, (gs, gn) in enumerate(GTS):
                nc.tensor.matmul(u_ps[:], wc_sb[:gn, gt, :], compT[:gn, gt, :],
                                 start=False, stop=(gt == NGT - 1))
            u_sb = sb.tile([H, BL], F32)
            nc.scalar.copy(u_sb[:], u_ps[:])

            # ppi row-sums (overlappable with everything above)
            prs = sb.tile([128, NGT], F32)  # [p, gt]
            for gt, (gs, gn) in enumerate(GTS):
                ppi_sb = big.tile([128, G], F32, tag="ppi")
                nc.sync.dma_start(out=ppi_sb[:gn, :], in_=ppi[gs:gs + gn, :])
                if gt % 2 == 0:
                    nc.vector.tensor_reduce(prs[:gn, gt:gt + 1], ppi_sb[:gn, :],
                                            mybir.AxisListType.X, mybir.AluOpType.add)
                else:
                    nc.scalar.activation(ppi_sb[:gn, :], ppi_sb[:gn, :], AF.Copy,
                                         accum_out=prs[:gn, gt:gt + 1])

            # A/C, score-sum, pred (gene-major), LN stats
            stats_ps = pacc.tile([1, 2, BL], F32, tag="stats")
            predT = sb.tile([128, NGT, BL], F32)
            for gt, (gs, gn) in enumerate(GTS):
                wgT_ps = cyc([H, 128])
                nc.tensor.transpose(wgT_ps[:, :gn], wg_sb[:gn, gt, :], ident[:gn, :gn])
                wgT = big.tile([H, 128], F32, tag="wgT")
                nc.scalar.copy(wgT[:, :gn], wgT_ps[:, :gn])
                wcT_ps = cyc([H, 128])
                nc.tensor.transpose(wcT_ps[:, :gn], wc_sb[:gn, gt, :], ident[:gn, :gn])
                wcT = big.tile([H, 128], F32, tag="wcT")
                nc.scalar.copy(wcT[:, :gn], wcT_ps[:, :gn])

                A_ps = cyc([128, BL])
                nc.tensor.matmul(A_ps[:gn, :], wgT[:, :gn], u_sb[:], start=True, stop=True)
                C_ps = cyc([128, BL])
                nc.tensor.matmul(C_ps[:gn, :], wcT[:, :gn], u_sb[:], start=True, stop=True)

                t1 = sb.tile([128, BL], F32, tag="t1")
                nc.vector.tensor_mul(t1[:gn, :], bgT[:gn, gt, :], A_ps[:gn, :])
                t2 = sb.tile([128, BL], F32, tag="t2")
                nc.vector.tensor_mul(t2[:gn, :], compT[:gn, gt, :], C_ps[:gn, :])
                nc.vector.tensor_add(t1[:gn, :], t1[:gn, :], t2[:gn, :])
                # (ssum*1/sqrt(H) + prs) ; then pred = b_gex * that
                nc.vector.tensor_scalar(t1[:gn, :], t1[:gn, :],
                                        inv_sqrt_h, prs[:gn, gt:gt + 1],
                                        op0=mybir.AluOpType.mult,
                                        op1=mybir.AluOpType.add)
                nc.vector.tensor_mul(predT[:gn, gt, :], bgT[:gn, gt, :], t1[:gn, :])

                st = sb.tile([128, 2, BL], F32, tag="st")
                nc.scalar.copy(st[:gn, 0, :], predT[:gn, gt, :])
                nc.vector.tensor_mul(st[:gn, 1, :], predT[:gn, gt, :], predT[:gn, gt, :])
                nc.tensor.matmul(stats_ps[:].rearrange("o s b -> o (s b)"),
                                 ones_col[:gn, :],
                                 st[:gn, :, :].rearrange("p s b -> p (s b)"),
                                 start=(gt == 0), stop=(gt == NGT - 1))

            # ================= LayerNorm + ReLU =================
            mu = sb.tile([1, BL], F32)
            nc.vector.tensor_scalar_mul(mu[:], stats_ps[:, 0, :], 1.0 / G)
            ex2 = sb.tile([1, BL], F32)
            nc.vector.tensor_scalar_mul(ex2[:], stats_ps[:, 1, :], 1.0 / G)
            mu2 = sb.tile([1, BL], F32)
            nc.vector.tensor_mul(mu2[:], mu[:], mu[:])
            var = sb.tile([1, BL], F32)
            nc.vector.tensor_sub(var[:], ex2[:], mu2[:])
            sd = sb.tile([1, BL], F32)
            nc.scalar.activation(sd[:], var[:], AF.Sqrt, bias=eps_t[:1, 0:1])
            rstd = sb.tile([1, BL], F32)
            nc.vector.reciprocal(rstd[:], sd[:])
            mr = sb.tile([1, 2, BL], F32)
            nc.vector.tensor_copy(mr[:, 0, :], mu[:])
            nc.vector.tensor_copy(mr[:, 1, :], rstd[:])
            mr_ps = cyc([128, 2 * BL])
            nc.tensor.matmul(mr_ps[:], ones_row[:], mr[:].rearrange("o s b -> o (s b)"),
                             start=True, stop=True)
            mr_b = mr_ps[:].rearrange("p (s b) -> p s b", s=2)

            gam = sb.tile([128, NGT], F32)
            bet = sb.tile([128, NGT], F32)
            for gt, (gs, gn) in enumerate(GTS):
                nc.sync.dma_start(out=gam[:gn, gt:gt + 1],
                                  in_=ln_gamma[gs:gs + gn].rearrange("(g o) -> g o", o=1))
                nc.sync.dma_start(out=bet[:gn, gt:gt + 1],
                                  in_=ln_beta[gs:gs + gn].rearrange("(g o) -> g o", o=1))

            xn = sb.tile([128, NGT, BL], F32)
            for gt, (gs, gn) in enumerate(GTS):
                xm = sb.tile([128, BL], F32, tag="xm")
                nc.vector.tensor_sub(xm[:gn, :], predT[:gn, gt, :], mr_b[:gn, 0, :])
                nc.vector.tensor_mul(xm[:gn, :], xm[:gn, :], mr_b[:gn, 1, :])
                nc.scalar.activation(xn[:gn, gt, :], xm[:gn, :], AF.Relu,
                                     scale=gam[:gn, gt:gt + 1], bias=bet[:gn, gt:gt + 1])

            # ================= FFN =================
            NSPLIT = [(0, 512), (512, 466)]
            o_ps = [pacc.tile([BL, n], F32, tag=f"o{i}", name=f"o_ps{i}")
                    for i, (s, n) in enumerate(NSPLIT)]
            for kt, (ks, kn) in enumerate(GTS):
                wff_sb = big.tile([128, G], F32, tag="wff")
                nc.sync.dma_start(out=wff_sb[:kn, :], in_=W_ff[ks:ks + kn, :])
                for i, (ns, nn) in enumerate(NSPLIT):
                    nc.tensor.matmul(o_ps[i][:], xn[:kn, kt, :], wff_sb[:kn, ns:ns + nn],
                                     start=(kt == 0), stop=(kt == NGT - 1))
            pred_out = sb.tile([BL, G], F32)
            for i, (ns, nn) in enumerate(NSPLIT):
                nc.scalar.copy(pred_out[:, ns:ns + nn], o_ps[i][:])
            nc.sync.dma_start(out=out_pred[:, :], in_=pred_out[:])

    _split_excess_waits(nc)
    return nc


_PER_SAMPLE = ("b_gex", "node_feat", "mask", "adj_matrix", "dist_matrix", "dose", "time")


def kernel(**inputs):
    inputs = {k: np.ascontiguousarray(np.asarray(v, dtype=np.float32))
              for k, v in inputs.items()}
    nc = build_nc()
    in_maps = []
    for c in range(NCORES):
        m = {}
        for k, v in inputs.items():
            if k in _PER_SAMPLE:
                m[k] = np.ascontiguousarray(v[c * BL:(c + 1) * BL])
            else:
                m[k] = v
        in_maps.append(m)
    r = run_bass_kernel_spmd(nc, in_maps, list(range(NCORES)))
    pred = np.concatenate([r.results[c]["out_pred"] for c in range(NCORES)], axis=0)
    comp = np.concatenate([r.results[c]["out_comp"] for c in range(NCORES)], axis=0)
    return pred, comp


# revision 4
# speedup vs baseline: 1.4691x; 1.4691x over previous
"""Trainium2 Bass kernel for nn_CSG2A_net (gnn_message_passing).

Math (algebraically identical to the reference, never materializes the
[B,G,G] score tensor):
  CCE:  h = relu(node_feat @ W1); w = adj*exp(-dist)
        g[b,m] = sum_n mask[b,n] * w[b,n,m]
        pooled[b,d] = (sum_m g[b,m] h[b,m,d]) / clip(sum_n mask[b,n], 1)
        comp = pooled @ W2 + dose @ w_dose + time @ w_time
  score.sum(-1)[b,g] = q[b,g,:] . (sum_k q[b,k,:]) / sqrt(H)
    with q[b,g,:] = b_gex[b,g] w_gex[g,:] + comp[b,g] w_comp[g,:]
    so  u = b_gex @ w_gex + comp @ w_comp          [B,H]
        A = u @ w_gex.T ; C = u @ w_comp.T         [B,G]
        ssum = (b_gex*A + comp*C) / sqrt(H)
  pred = b_gex * (ssum + ppi_adj.sum(-1))
  out  = relu(LN(pred)) @ W_ff

Sharding: data-parallel over batch across 8 cores (8 samples each);
weights replicated.  On-chip layout is gene-major ([G-tile partitions x
batch free]) so every matmul contracts on the partition dim.

DMA strategy (cost-model driven): big contiguous weight loads ride
HWDGE on the sync engine (transfer-bound, pipelined); small/strided
loads ride SWDGE on the idle gpsimd engine; b_gex is loaded naturally
and transposed on the PE instead of a 4B-gather DMA.  FFN matmuls run
as float32r (TF32-like) for 4x PE throughput.
"""

import numpy as np

import concourse.bass as bass
import concourse.mybir as mybir
import concourse.tile as tile
from concourse.bass_utils import run_bass_kernel_spmd
from concourse.masks import make_identity

F32 = mybir.dt.float32
F32R = mybir.dt.float32r
AF = mybir.ActivationFunctionType

G, H, NA, FEAT, CH = 978, 128, 50, 34, 64
B, NCORES = 64, 8
BL = B // NCORES  # per-core batch
LN_EPS = 1e-5
# gene-dim tiles: 7 x 128 + 82
GTS = [(i * 128, 128) for i in range(7)] + [(896, 82)]
NGT = len(GTS)

_DMA_ZERO_WAIT = ("InstDMACopy", "InstDMATransposeAnt", "InstTriggeredCopy")


def _split_excess_waits(nc):
    """walrus in this container accepts at most 1 inline sync-wait per
    instruction (0 for DMA).  Move excess waits onto same-engine nops
    inserted immediately before the overloaded instruction."""

    def make_nop(engine):
        bi = nc.engines[engine].nop(nofuse=True)
        ins = bi.ins
        lst = nc.cur_bb.bb.instructions
        assert lst[-1] is ins
        lst.pop()
        return ins

    for bb in nc.main_func.blocks:
        lst = bb.instructions
        i = 0
        while i < len(lst):
            ins = lst[i]
            si = getattr(ins, "sync_info", None)
            waits = list(si.on_wait) if (si and si.on_wait) else []
            limit = 0 if type(ins).__name__ in _DMA_ZERO_WAIT else 1
            if len(waits) > limit:
                keep = waits[len(waits) - limit:] if limit else []
                excess = waits[: len(waits) - limit]
                si.on_wait = keep
                pos = i
                for w in excess:
                    nop = make_nop(ins.engine)
                    nop.sync_info = mybir.SyncInfo(on_wait=[w], on_update=[])
                    lst.insert(pos, nop)
                    pos += 1
                    i += 1
            i += 1


def build_nc():
    nc = bass.Bass()

    # ---- kernel I/O (per-core shapes) ----
    b_gex = nc.dram_tensor("b_gex", [BL, G], F32, kind="ExternalInput")
    node_feat = nc.dram_tensor("node_feat", [BL, NA, FEAT], F32, kind="ExternalInput")
    mask = nc.dram_tensor("mask", [BL, NA], F32, kind="ExternalInput")
    adj = nc.dram_tensor("adj_matrix", [BL, NA, NA], F32, kind="ExternalInput")
    dist = nc.dram_tensor("dist_matrix", [BL, NA, NA], F32, kind="ExternalInput")
    dose = nc.dram_tensor("dose", [BL, 1], F32, kind="ExternalInput")
    time_in = nc.dram_tensor("time", [BL, 1], F32, kind="ExternalInput")
    ppi = nc.dram_tensor("ppi_adj", [G, G], F32, kind="ExternalInput")
    w_gex = nc.dram_tensor("w_gex", [G, H], F32, kind="ExternalInput")
    w_comp = nc.dram_tensor("w_comp", [G, H], F32, kind="ExternalInput")
    W1 = nc.dram_tensor("W1", [FEAT, CH], F32, kind="ExternalInput")
    W2 = nc.dram_tensor("W2", [CH, G], F32, kind="ExternalInput")
    w_dose = nc.dram_tensor("w_dose", [1, G], F32, kind="ExternalInput")
    w_time = nc.dram_tensor("w_time", [1, G], F32, kind="ExternalInput")
    ln_gamma = nc.dram_tensor("ln_gamma", [G], F32, kind="ExternalInput")
    ln_beta = nc.dram_tensor("ln_beta", [G], F32, kind="ExternalInput")
    W_ff = nc.dram_tensor("W_ff", [G, G], F32, kind="ExternalInput")

    out_pred = nc.dram_tensor("out_pred", [BL, G], F32, kind="ExternalOutput")
    out_comp = nc.dram_tensor("out_comp", [BL, G], F32, kind="ExternalOutput")

    inv_sqrt_h = 1.0 / float(np.sqrt(H))

    with tile.TileContext(nc) as tc:
        with (
            tc.tile_pool(name="const", bufs=1) as const,
            tc.tile_pool(name="sb", bufs=1) as sb,
            tc.tile_pool(name="big", bufs=3) as big,
            tc.tile_pool(name="pacc", bufs=1, space="PSUM") as pacc,
            tc.tile_pool(name="pcyc", bufs=4, space="PSUM") as pcyc,
        ):
            ident = const.tile([128, 128], F32)
            make_identity(nc, ident[:])
            ones_col = const.tile([128, 1], F32)   # lhsT for col-sums
            nc.vector.memset(ones_col[:], 1.0)
            ones_row = const.tile([1, 128], F32)   # lhsT for partition-bcast
            nc.vector.memset(ones_row[:], 1.0)
            eps_t = const.tile([1, 1], F32)
            nc.vector.memset(eps_t[:], LN_EPS)

            _cyc_n = [0]

            def cyc(shape):
                _cyc_n[0] += 1
                return pcyc.tile(shape, F32, tag="cyc", name=f"cyc{_cyc_n[0]}")

            # ============ small loads on gpsimd (SWDGE) ============
            nfT = sb.tile([FEAT, BL, NA], F32)
            nc.gpsimd.dma_start(out=nfT[:], in_=node_feat[:, :, :].rearrange("b n f -> f b n"))
            W1_sb = sb.tile([FEAT, CH], F32)
            nc.gpsimd.dma_start(out=W1_sb[:], in_=W1[:, :])
            adjT = sb.tile([NA, BL, NA], F32)
            nc.gpsimd.dma_start(out=adjT[:], in_=adj[:, :, :].rearrange("b n m -> n b m"))
            distT = sb.tile([NA, BL, NA], F32)
            nc.gpsimd.dma_start(out=distT[:], in_=dist[:, :, :].rearrange("b n m -> n b m"))
            maskT = sb.tile([NA, BL], F32)
            nc.gpsimd.dma_start(out=maskT[:], in_=mask[:, :].rearrange("b n -> n b"))
            doseT = sb.tile([1, BL], F32)
            nc.gpsimd.dma_start(out=doseT[:], in_=dose[:, :].rearrange("b o -> o b"))
            timeT = sb.tile([1, BL], F32)
            nc.gpsimd.dma_start(out=timeT[:], in_=time_in[:, :].rearrange("b o -> o b"))
            wdose_sb = sb.tile([1, G], F32)
            nc.gpsimd.dma_start(out=wdose_sb[:], in_=w_dose[:, :])
            wtime_sb = sb.tile([1, G], F32)
            nc.gpsimd.dma_start(out=wtime_sb[:], in_=w_time[:, :])
            gam = sb.tile([128, NGT], F32)
            nc.gpsimd.dma_start(out=gam[:, :7],
                                in_=ln_gamma[0:896].rearrange("(t p) -> p t", p=128))
            nc.gpsimd.dma_start(out=gam[:82, 7:8],
                                in_=ln_gamma[896:G].rearrange("(p o) -> p o", o=1))
            bet = sb.tile([128, NGT], F32)
            nc.gpsimd.dma_start(out=bet[:, :7],
                                in_=ln_beta[0:896].rearrange("(t p) -> p t", p=128))
            nc.gpsimd.dma_start(out=bet[:82, 7:8],
                                in_=ln_beta[896:G].rearrange("(p o) -> p o", o=1))

            # ============ big loads on sync (HWDGE) ============
            W2_sb = sb.tile([CH, G], F32)
            nc.sync.dma_start(out=W2_sb[:], in_=W2[:, :])
            b_nat = sb.tile([BL, G], F32)
            nc.sync.dma_start(out=b_nat[:], in_=b_gex[:, :])
            wg_sb = sb.tile([128, NGT, H], F32)
            nc.sync.dma_start(out=wg_sb[:, :7, :],
                              in_=w_gex[0:896, :].rearrange("(t p) h -> p t h", p=128))
            nc.sync.dma_start(out=wg_sb[:82, 7, :], in_=w_gex[896:G, :])
            wc_sb = sb.tile([128, NGT, H], F32)
            nc.sync.dma_start(out=wc_sb[:, :7, :],
                              in_=w_comp[0:896, :].rearrange("(t p) h -> p t h", p=128))
            nc.sync.dma_start(out=wc_sb[:82, 7, :], in_=w_comp[896:G, :])

            # ppi row-sums (consumed mid-kernel)
            prs = sb.tile([128, NGT], F32)  # [p, gt]
            for gt, (gs, gn) in enumerate(GTS):
                ppi_sb = big.tile([128, G], F32, tag="ppi", name=f"ppi_sb{gt}")
                nc.sync.dma_start(out=ppi_sb[:gn, :], in_=ppi[gs:gs + gn, :])
                if gt % 2 == 0:
                    nc.vector.tensor_reduce(prs[:gn, gt:gt + 1], ppi_sb[:gn, :],
                                            mybir.AxisListType.X, mybir.AluOpType.add)
                else:
                    nc.scalar.activation(ppi_sb[:gn, :], ppi_sb[:gn, :], AF.Copy,
                                         accum_out=prs[:gn, gt:gt + 1])

            # W_ff: fully resident so the DMA chain never stalls on FFN pace
            wff_sb = sb.tile([128, NGT, G], F32)
            for kt, (ks, kn) in enumerate(GTS):
                nc.sync.dma_start(out=wff_sb[:kn, kt, :], in_=W_ff[ks:ks + kn, :])

            # b_gex transposed to gene-major via PE (avoids 4B-gather DMA)
            bgT = sb.tile([128, NGT, BL], F32)
            for gt, (gs, gn) in enumerate(GTS):
                bg_ps = cyc([128, BL])
                nc.tensor.transpose(bg_ps[:gn, :], b_nat[:, gs:gs + gn], ident[:BL, :BL])
                nc.scalar.copy(bgT[:gn, gt, :], bg_ps[:gn, :])

            # ================= CCE =================
            hT_ps = cyc([CH, BL * NA])
            nc.tensor.matmul(hT_ps[:], W1_sb[:], nfT[:].rearrange("f b n -> f (b n)"),
                             start=True, stop=True)
            hT = sb.tile([CH, BL, NA], F32)
            nc.scalar.activation(hT[:].rearrange("d b n -> d (b n)"), hT_ps[:], AF.Relu)

            wmsg = sb.tile([NA, BL, NA], F32)
            nc.scalar.activation(wmsg[:], distT[:], AF.Exp, scale=-1.0)
            nc.vector.tensor_mul(wmsg[:], wmsg[:], adjT[:])

            g_ps = cyc([1, BL * NA])
            for b in range(BL):
                nc.tensor.matmul(g_ps[:, b * NA:(b + 1) * NA],
                                 maskT[:, b:b + 1], wmsg[:, b, :],
                                 start=True, stop=True)
            gb_ps = cyc([CH, BL * NA])
            g_sb = sb.tile([1, BL * NA], F32)
            nc.vector.tensor_copy(g_sb[:], g_ps[:])
            nc.tensor.matmul(gb_ps[:], ones_row[:1, :CH], g_sb[:], start=True, stop=True)

            prod = sb.tile([CH, BL, NA], F32)
            nc.vector.tensor_mul(prod[:].rearrange("d b n -> d (b n)"),
                                 hT[:].rearrange("d b n -> d (b n)"), gb_ps[:])
            pooled_raw = sb.tile([CH, BL], F32)
            nc.vector.tensor_reduce(pooled_raw[:], prod[:], mybir.AxisListType.X,
                                    mybir.AluOpType.add)

            ms_ps = cyc([1, BL])
            nc.tensor.matmul(ms_ps[:], ones_col[:NA, :], maskT[:], start=True, stop=True)
            ms_sb = sb.tile([1, BL], F32)
            nc.vector.tensor_scalar_max(ms_sb[:], ms_ps[:], 1.0)
            rms = sb.tile([1, BL], F32)
            nc.vector.reciprocal(rms[:], ms_sb[:])
            rb_ps = cyc([CH, BL])
            nc.tensor.matmul(rb_ps[:], ones_row[:1, :CH], rms[:], start=True, stop=True)
            pooledT = sb.tile([CH, BL], F32)
            nc.vector.tensor_mul(pooledT[:], pooled_raw[:], rb_ps[:])

            # comp.T per gene tile (+ comp output)
            compT = sb.tile([128, NGT, BL], F32)  # [p, gt, b]
            comp_out = sb.tile([BL, G], F32)
            for gt, (gs, gn) in enumerate(GTS):
                cT_ps = cyc([128, BL])
                nc.tensor.matmul(cT_ps[:gn, :], W2_sb[:, gs:gs + gn], pooledT[:],
                                 start=True, stop=False)
                nc.tensor.matmul(cT_ps[:gn, :], wdose_sb[:1, gs:gs + gn], doseT[:],
                                 start=False, stop=False)
                nc.tensor.matmul(cT_ps[:gn, :], wtime_sb[:1, gs:gs + gn], timeT[:],
                                 start=False, stop=True)
                nc.scalar.copy(compT[:gn, gt, :], cT_ps[:gn, :])
                c8_ps = cyc([BL, 128])
                nc.tensor.transpose(c8_ps[:, :gn], compT[:gn, gt, :], ident[:gn, :gn])
                nc.scalar.copy(comp_out[:, gs:gs + gn], c8_ps[:, :gn])
            nc.sync.dma_start(out=out_comp[:, :], in_=comp_out[:])

            # ================= attention-sum =================
            u_ps = pacc.tile([H, BL], F32, tag="u")
            for gt, (gs, gn) in enumerate(GTS):
                nc.tensor.matmul(u_ps[:], wg_sb[:gn, gt, :], bgT[:gn, gt, :],
                                 start=(gt == 0), stop=False)
            for gt, (gs, gn) in enumerate(GTS):
                nc.tensor.matmul(u_ps[:], wc_sb[:gn, gt, :], compT[:gn, gt, :],
                                 start=False, stop=(gt == NGT - 1))
            u_sb = sb.tile([H, BL], F32)
            nc.scalar.copy(u_sb[:], u_ps[:])

            # A/C, score-sum, pred (gene-major), LN stats
            stats_ps = pacc.tile([1, 2, BL], F32, tag="stats")
            predT = sb.tile([128, NGT, BL], F32)
            for gt, (gs, gn) in enumerate(GTS):
                wgT_ps = cyc([H, 128])
                nc.tensor.transpose(wgT_ps[:, :gn], wg_sb[:gn, gt, :], ident[:gn, :gn])
                wgT = big.tile([H, 128], F32, tag="wgT")
                nc.scalar.copy(wgT[:, :gn], wgT_ps[:, :gn])
                wcT_ps = cyc([H, 128])
                nc.tensor.transpose(wcT_ps[:, :gn], wc_sb[:gn, gt, :], ident[:gn, :gn])
                wcT = big.tile([H, 128], F32, tag="wcT")
                nc.scalar.copy(wcT[:, :gn], wcT_ps[:, :gn])

                A_ps = cyc([128, BL])
                nc.tensor.matmul(A_ps[:gn, :], wgT[:, :gn], u_sb[:], start=True, stop=True)
                C_ps = cyc([128, BL])
                nc.tensor.matmul(C_ps[:gn, :], wcT[:, :gn], u_sb[:], start=True, stop=True)

                t1 = sb.tile([128, BL], F32, tag="t1")
                nc.vector.tensor_mul(t1[:gn, :], bgT[:gn, gt, :], A_ps[:gn, :])
                t2 = sb.tile([128, BL], F32, tag="t2")
                nc.vector.tensor_mul(t2[:gn, :], compT[:gn, gt, :], C_ps[:gn, :])
                nc.vector.tensor_add(t1[:gn, :], t1[:gn, :], t2[:gn, :])
                # (ssum*1/sqrt(H) + prs) ; then pred = b_gex * that
                nc.vector.tensor_scalar(t1[:gn, :], t1[:gn, :],
                                        inv_sqrt_h, prs[:gn, gt:gt + 1],
                                        op0=mybir.AluOpType.mult,
                                        op1=mybir.AluOpType.add)
                nc.vector.tensor_mul(predT[:gn, gt, :], bgT[:gn, gt, :], t1[:gn, :])

                st = sb.tile([128, 2, BL], F32, tag="st")
                nc.scalar.copy(st[:gn, 0, :], predT[:gn, gt, :])
                nc.vector.tensor_mul(st[:gn, 1, :], predT[:gn, gt, :], predT[:gn, gt, :])
                nc.tensor.matmul(stats_ps[:].rearrange("o s b -> o (s b)"),
                                 ones_col[:gn, :],
                                 st[:gn, :, :].rearrange("p s b -> p (s b)"),
                                 start=(gt == 0), stop=(gt == NGT - 1))

            # ================= LayerNorm + ReLU =================
            mu = sb.tile([1, BL], F32)
            nc.vector.tensor_scalar_mul(mu[:], stats_ps[:, 0, :], 1.0 / G)
            ex2 = sb.tile([1, BL], F32)
            nc.vector.tensor_scalar_mul(ex2[:], stats_ps[:, 1, :], 1.0 / G)
            mu2 = sb.tile([1, BL], F32)
            nc.vector.tensor_mul(mu2[:], mu[:], mu[:])
            var = sb.tile([1, BL], F32)
            nc.vector.tensor_sub(var[:], ex2[:], mu2[:])
            sd = sb.tile([1, BL], F32)
            nc.scalar.activation(sd[:], var[:], AF.Sqrt, bias=eps_t[:1, 0:1])
            rstd = sb.tile([1, BL], F32)
            nc.vector.reciprocal(rstd[:], sd[:])
            mr = sb.tile([1, 2, BL], F32)
            nc.vector.tensor_copy(mr[:, 0, :], mu[:])
            nc.vector.tensor_copy(mr[:, 1, :], rstd[:])
            mr_ps = cyc([128, 2 * BL])
            nc.tensor.matmul(mr_ps[:], ones_row[:], mr[:].rearrange("o s b -> o (s b)"),
                             start=True, stop=True)
            mr_b = mr_ps[:].rearrange("p (s b) -> p s b", s=2)

            xn = sb.tile([128, NGT, BL], F32)
            for gt, (gs, gn) in enumerate(GTS):
                xm = sb.tile([128, BL], F32, tag="xm")
                nc.vector.tensor_sub(xm[:gn, :], predT[:gn, gt, :], mr_b[:gn, 0, :])
                nc.vector.tensor_mul(xm[:gn, :], xm[:gn, :], mr_b[:gn, 1, :])
                nc.scalar.activation(xn[:gn, gt, :], xm[:gn, :], AF.Relu,
                                     scale=gam[:gn, gt:gt + 1], bias=bet[:gn, gt:gt + 1])

            # ================= FFN (float32r for 4x PE rate) =================
            NSPLIT = [(0, 512), (512, 466)]
            o_ps = [pacc.tile([BL, n], F32, tag=f"o{i}", name=f"o_ps{i}")
                    for i, (s, n) in enumerate(NSPLIT)]
            for kt, (ks, kn) in enumerate(GTS):
                for i, (ns, nn) in enumerate(NSPLIT):
                    nc.tensor.matmul(o_ps[i][:],
                                     xn[:kn, kt, :].bitcast(F32R),
                                     wff_sb[:kn, kt, ns:ns + nn].bitcast(F32R),
                                     start=(kt == 0), stop=(kt == NGT - 1))
            pred_out = sb.tile([BL, G], F32)
            for i, (ns, nn) in enumerate(NSPLIT):
                nc.scalar.copy(pred_out[:, ns:ns + nn], o_ps[i][:])
            nc.sync.dma_start(out=out_pred[:, :], in_=pred_out[:])

    _split_excess_waits(nc)
    return nc


_PER_SAMPLE = ("b_gex", "node_feat", "mask", "adj_matrix", "dist_matrix", "dose", "time")


def kernel(**inputs):
    inputs = {k: np.ascontiguousarray(np.asarray(v, dtype=np.float32))
              for k, v in inputs.items()}
    nc = build_nc()
    in_maps = []
    for c in range(NCORES):
        m = {}
        for k, v in inputs.items():
            if k in _PER_SAMPLE:
                m[k] = np.ascontiguousarray(v[c * BL:(c + 1) * BL])
            else:
                m[k] = v
        in_maps.append(m)
    r = run_bass_kernel_spmd(nc, in_maps, list(range(NCORES)))
    pred = np.concatenate([r.results[c]["out_pred"] for c in range(NCORES)], axis=0)
    comp = np.concatenate([r.results[c]["out_comp"] for c in range(NCORES)], axis=0)
    return pred, comp
